# revision 19
# baseline (speedup 1.0000x reference)
"""Trainium2 Bass kernel: single-head causal self-attention.

Problem: x[B=8, S=2048, D=1024], Wq/Wk/Wv[D, H=64], bq/bk/bv[H].
    q = x@Wq+bq; k = x@Wk+bk; v = x@Wv+bv
    out = softmax(causal(q k^T) / sqrt(H)) @ v

Sharding: batch -> 8 NeuronCores (data parallel, no collectives).

Per-core strategy (v4; baseline ~68us):
  - all matmuls bf16; scores built transposed: S^T [128 k, 512 q] =
    K_i Q^T in PSUM; exp (scale=1/8, bf16 out) gives P^T tiles = the
    moving operand of O^T = (V|1)^T P^T; the (V|1) ones-column yields
    the softmax denominator as an extra output row
  - head: first matmul gated on ~113 KiB across 3 HW queues; pieces
    issued in need-order alternating sync/scalar (HWDGE only; all
    concurrent transfers share ~190 GB/s, so piece count is kept low
    to get the J1 triggers out early); 1 MiB J-chunks split in half
    across the two engines
  - att is exp(scalar)-latency-bound: PV pairs lag one pair behind
    scores; proj(J+1) piece closures (qk c0-7, qk-finish, vv c0-7)
    interleave 2-per-pair into att(J)'s exp gaps; qk-finish (bias +
    K^T move) sits between qk and vv so KT0 is ready early
  - causal mask: fully-masked q-ranges trimmed from matmuls AND exp;
    the 128x128 triangle gets -1e30 DVE-added on PSUM (off the PE)
  - K^T to partitions 0:64: PE selector (shared vtr/ksel bank) +
    scalar copy for J=0,1; SBUF->SBUF sync DMA for J=2,3
  - deps are tile-granular, so J3 accumulates O^T in two tiles:
    cols [0,256) in the "ot" bank (group closes at PV i=13) and cols
    [256,512) in the idle proj ring (closes at i=15) -> the first
    half normalizes + stores while the last PV pair still runs;
    half-stores go out on sync and scalar
"""

import sys

sys.path.insert(0, "/opt/trn_rl_repo")

import numpy as np

B, S, D, H = 8, 2048, 1024, 64
N_CORES = 8
SQ = 512            # q chunk (PSUM bank / fp32 moving max)
NQ = S // SQ        # 4
ND = D // 128       # 8 contraction chunks for projections
H1 = H + 1          # V plus ones column
CW = 2 * H + H1     # packed weight cols per chunk: wq|wk|wv1 = 193
TRIM = True         # skip fully-masked q-ranges in diagonal blocks

_CACHE = {}


def _build_nc():
    import concourse.tile as tile
    from concourse import bacc, mybir

    f32 = mybir.dt.float32
    bf16 = mybir.dt.bfloat16
    AF = mybir.ActivationFunctionType
    ALU = mybir.AluOpType

    nc = bacc.Bacc(None, target_bir_lowering=False)
    xTp = nc.dram_tensor("xTp", [NQ, 128, ND * SQ], bf16, kind="ExternalInput")
    # wqkv: [bqk | bv1 | chunk c: (wq|wk 128 cols, wv1 65 cols)]
    wqkv = nc.dram_tensor("wqkv", [128, 2 + ND * CW], bf16, kind="ExternalInput")
    # idt: [identb(128) | tneg(128)]
    idt = nc.dram_tensor("idt", [128, 256], bf16, kind="ExternalInput")
    out = nc.dram_tensor("out", [S, H], f32, kind="ExternalOutput")

    with tile.TileContext(nc) as tc:
        from contextlib import ExitStack

        with ExitStack() as ctx:
            const = ctx.enter_context(tc.tile_pool(name="const", bufs=1))
            sb = ctx.enter_context(tc.tile_pool(name="sb", bufs=1))
            pt_pool = ctx.enter_context(tc.tile_pool(name="pt", bufs=6))
            o_pool = ctx.enter_context(tc.tile_pool(name="o", bufs=2))
            ps = ctx.enter_context(tc.tile_pool(name="ps", bufs=1, space="PSUM"))

            wq_sb = const.tile([128, 2 + ND * CW], bf16)
            idt_sb = const.tile([128, 256], bf16)
            identb_sb = idt_sb[:, 0:128]
            tneg_sb = idt_sb[:, 128:256]

            def wchunk(c):       # wq|wk stationary for chunk c
                return wq_sb[:, 2 + c * CW : 2 + c * CW + 2 * H]

            def vchunk(c):       # wv|ones-bias stationary for chunk c
                return wq_sb[:, 2 + c * CW + 2 * H : 2 + (c + 1) * CW]

            bf = const.tile([128, 2], f32)
            bqk_sb = bf[:, 0:1]
            bv1_sb = bf[:H1, 1:2]

            xt = {}          # J -> [128, ND*SQ] bf16 (c-chunk at cols c*SQ)
            QKT = {}         # J -> [128, SQ] bf16 (Q^T rows 0:64, K^T 64:128)
            KT0 = {}         # J -> [64, SQ] bf16 at base partition 0
            Vones = {}       # J -> [128, 4*66] bf16 ((V|1) rows, stride 66)
            VT1 = {}         # J -> [H1, SQ] bf16 (V^T plus ones row)
            OT = {}

            # ---- DMA issue order: need-ordered, low piece count ----
            with nc.named_scope("load"):
                for J in range(NQ):
                    t_x = sb.tile([128, ND * SQ], bf16, tag=f"x{J}")
                    xt[J] = t_x
                x0 = xt[0]
                HP = ND * SQ // 2   # half of a J chunk
                nc.sync.dma_start(wq_sb[:, : 2 + CW], wqkv[:, : 2 + CW])
                nc.scalar.dma_start(x0[:, 256:512], xTp[0, :, 256:512])
                nc.sync.dma_start(x0[:, 0:256], xTp[0, :, 0:256])
                nc.scalar.dma_start(x0[:, 512:1024], xTp[0, :, 512:1024])
                nc.sync.dma_start(
                    wq_sb[:, 2 + CW : 2 + 4 * CW], wqkv[:, 2 + CW : 2 + 4 * CW]
                )
                nc.sync.dma_start(x0[:, 1024:2048], xTp[0, :, 1024:2048])
                nc.sync.dma_start(idt_sb[:], idt[:, :])
                nc.scalar.dma_start(x0[:, 2048:3072], xTp[0, :, 2048:3072])
                nc.sync.dma_start(x0[:, 3072:4096], xTp[0, :, 3072:4096])
                nc.scalar.dma_start(wq_sb[:, 2 + 4 * CW :], wqkv[:, 2 + 4 * CW :])
                nc.scalar.dma_start(xt[1][:, :HP], xTp[1, :, :HP])
                nc.sync.dma_start(xt[1][:, HP:], xTp[1, :, HP:])
                nc.scalar.dma_start(xt[2][:, :HP], xTp[2, :, :HP])
                nc.sync.dma_start(xt[2][:, HP:], xTp[2, :, HP:])
                nc.scalar.dma_start(xt[3][:, :HP], xTp[3, :, :HP])
                nc.sync.dma_start(xt[3][:, HP:], xTp[3, :, HP:])
                nc.vector.tensor_copy(bf[:], wq_sb[:, 0:2])

            def proj_start(J):
                """Alloc proj PSUM tiles; return (qk, vv, piece closures).
                Pieces: qk c0..c7, qk-finish (bias + K^T move), vv c0..c7.
                Emitting them in order (possibly interleaved into att)
                reproduces the projection."""
                qk = ps.tile([128, SQ], f32, tag="proj", bufs=2)
                vv = ps.tile([H1, SQ], f32, tag="proj", bufs=2)

                def mk_qk(c):
                    def em():
                        if J == 0 and c == 0:
                            nc.tensor.matmul(
                                qk[:, 0:256], wchunk(0), xt[0][:, 0:256],
                                start=True, stop=False,
                            )
                            # bytes still pending-zero from the first piece
                            nc.tensor.matmul(
                                qk[:, 256:512], wchunk(0), xt[0][:, 256:512],
                                start=False, stop=False,
                            )
                        else:
                            nc.tensor.matmul(
                                qk[:], wchunk(c),
                                xt[J][:, c * SQ : (c + 1) * SQ],
                                start=(c == 0), stop=(c == ND - 1),
                            )
                    return em

                def fin_qk():
                    with nc.named_scope(f"projf{J}"):
                        qkt = sb.tile([128, SQ], bf16, tag=f"qkt{J}")
                        nc.vector.tensor_scalar_add(
                            qkt[H:, :], qk[H:, :], bqk_sb[H:, :]
                        )
                        nc.vector.tensor_scalar_add(
                            qkt[:H, :], qk[:H, :], bqk_sb[:H, :]
                        )
                        QKT[J] = qkt
                        kt0 = sb.tile([H, SQ], bf16, tag=f"kt0{J}")
                        if J < 2:
                            ksel = ps.tile([H, SQ], f32, tag="vtr", bufs=1)
                            nc.tensor.matmul(
                                ksel[:], identb_sb[H:, H:], qkt[H:, :],
                                start=True, stop=True,
                            )
                            nc.scalar.activation(kt0[:], ksel[:], AF.Copy)
                        else:
                            nc.sync.dma_start(kt0[:], qkt[H:, :])
                        KT0[J] = kt0

                def mk_vv(c):
                    def em():
                        nc.tensor.matmul(
                            vv[:], vchunk(c), xt[J][:, c * SQ : (c + 1) * SQ],
                            start=(c == 0), stop=(c == ND - 1),
                        )
                    return em

                pieces = [mk_qk(c) for c in range(ND)]
                pieces.append(fin_qk)
                pieces += [mk_vv(c) for c in range(ND)]
                return qk, vv, pieces

            def proj_finish(J, vv):
                with nc.named_scope(f"projv{J}"):
                    vt1 = sb.tile([H1, SQ], bf16, tag=f"vt1{J}")
                    nc.vector.tensor_scalar_add(vt1[:], vv[:], bv1_sb[:])
                    VT1[J] = vt1

            def vtrans(J):
                with nc.named_scope(f"vtr{J}"):
                    vt1 = VT1[J]
                    pst = ps.tile([128, 4 * 66], bf16, tag="vtr", bufs=1)
                    for tt in range(4):
                        nc.tensor.transpose(
                            pst[:, tt * 66 : tt * 66 + H1],
                            vt1[:, tt * 128 : (tt + 1) * 128],
                            identb_sb[:H1, :H1],
                        )
                    vo = sb.tile([128, 4 * 66], bf16, tag=f"vo{J}")
                    nc.vector.tensor_copy(
                        vo[:].rearrange("p (t u) -> p t u", t=4)[:, :, 0:H1],
                        pst[:].rearrange("p (t u) -> p t u", t=4)[:, :, 0:H1],
                    )
                    Vones[J] = vo

            def att(J, inter=(), after_pair1=None):
                """PV pairs lag one pair behind scores; `inter` closures
                (next proj's pieces) are emitted 2 per pair from pair 1
                on, remainder after the loop."""
                last = J == NQ - 1
                inter = list(inter)
                ip = 0
                with nc.named_scope(f"att{J}"):
                    ot = ps.tile([H1, SQ], f32, tag="ot", bufs=1)
                    OT[J] = ot
                    nhalf = 2 * (J + 1)

                    def pv_pair(ii, pt):
                        diag = ii >= 2 * J
                        for h2 in range(2):
                            i = 2 * ii + h2
                            r = i - 4 * J
                            q0 = 128 * r if (TRIM and diag and r > 0) else 0
                            vo = Vones[i // 4][:, (i % 4) * 66 : (i % 4) * 66 + H1]
                            nc.tensor.matmul(
                                ot[:, q0:], vo,
                                pt[:, h2 * SQ + q0 : (h2 + 1) * SQ],
                                start=(i == 0),
                                stop=(i == 4 * (J + 1) - 1),
                            )

                    def score_pair(ii):
                        diag = ii >= 2 * J
                        st = ps.tile([128, 2 * SQ], f32, tag="st", bufs=2)
                        for h2 in range(2):
                            i = 2 * ii + h2
                            r = i - 4 * J
                            q0 = 128 * r if (TRIM and diag and r > 0) else 0
                            nc.tensor.matmul(
                                st[:, h2 * SQ + q0 : (h2 + 1) * SQ],
                                KT0[i // 4][:, (i % 4) * 128 : (i % 4 + 1) * 128],
                                QKT[J][:H, q0:],
                                start=True,
                                stop=True,
                            )
                            if diag:
                                nc.vector.tensor_tensor(
                                    st[:, h2 * SQ + 128 * r :
                                       h2 * SQ + 128 * (r + 1)],
                                    st[:, h2 * SQ + 128 * r :
                                       h2 * SQ + 128 * (r + 1)],
                                    tneg_sb[:],
                                    ALU.add,
                                )
                        pt = pt_pool.tile([128, 2 * SQ], bf16, tag="pt")
                        if TRIM and diag and ii == 2 * J:
                            nc.scalar.activation(
                                pt[:, 0:512], st[:, 0:512], AF.Exp, scale=0.125
                            )
                            nc.scalar.activation(
                                pt[:, 640:1024], st[:, 640:1024],
                                AF.Exp, scale=0.125,
                            )
                        elif TRIM and diag and ii == 2 * J + 1:
                            nc.scalar.activation(
                                pt[:, 256:512], st[:, 256:512],
                                AF.Exp, scale=0.125,
                            )
                            nc.scalar.activation(
                                pt[:, 896:1024], st[:, 896:1024],
                                AF.Exp, scale=0.125,
                            )
                        else:
                            nc.scalar.activation(
                                pt[:], st[:], AF.Exp, scale=0.125
                            )
                        return pt

                    # batch two pairs per PE group (4 score matmuls, then
                    # 4 PV matmuls of the previous batch): halves the
                    # group-boundary LDWEIGHTS exposure; PVs lag a batch
                    pend = []
                    for b in range(nhalf // 2):
                        cur = []
                        for ii in (2 * b, 2 * b + 1):
                            cur.append((ii, score_pair(ii)))
                        for p in pend:
                            pv_pair(*p)
                        pend = cur
                        if b == 0 and after_pair1 is not None:
                            after_pair1()
                        if b >= 1:
                            for _ in range(4):
                                if ip < len(inter):
                                    inter[ip]()
                                    ip += 1
                    for p in pend:
                        pv_pair(*p)
                    while ip < len(inter):
                        inter[ip]()
                        ip += 1

            def outp(J):
                # normalize + store rows 512J..512J+511
                last = J == NQ - 1
                halves = [(0, 2), (2, 4)] if last else [(0, 4)]
                # all-vector: scalar stays exp-only in the att windows
                use_vec = True
                with nc.named_scope(f"out{J}"):
                    ots = sb.tile([H1, SQ], bf16, tag=f"ots{J}")
                    ob = o_pool.tile([128, 4 * H], f32, tag="ob")
                    po = ps.tile([128, 4 * 66], bf16, tag="vtr", bufs=1)
                    rc = o_pool.tile([128, 4], f32, tag="rc")
                    for bi, (t0_, t1_) in enumerate(halves):
                        src = OT[J][:, t0_ * 128 : t1_ * 128]
                        if use_vec:
                            nc.vector.tensor_copy(
                                ots[:, t0_ * 128 : t1_ * 128], src
                            )
                        else:
                            nc.scalar.activation(
                                ots[:, t0_ * 128 : t1_ * 128], src, AF.Copy
                            )
                        for tt in range(t0_, t1_):
                            nc.tensor.transpose(
                                po[:, tt * 66 : tt * 66 + H1],
                                ots[:, tt * 128 : (tt + 1) * 128],
                                identb_sb[:H1, :H1],
                            )
                        nc.vector.reciprocal(
                            rc[:, t0_:t1_],
                            po[:, t0_ * 66 + H : t1_ * 66 : 66],
                        )
                        for tt in range(t0_, t1_):
                            if use_vec:
                                nc.vector.tensor_scalar_mul(
                                    ob[:, tt * H : (tt + 1) * H],
                                    po[:, tt * 66 : tt * 66 + H],
                                    rc[:, tt : tt + 1],
                                )
                            else:
                                nc.scalar.activation(
                                    ob[:, tt * H : (tt + 1) * H],
                                    po[:, tt * 66 : tt * 66 + H],
                                    AF.Copy,
                                    scale=rc[:, tt : tt + 1],
                                )
                        eng = nc.scalar if (last and bi == 1) else nc.sync
                        eng.dma_start(
                            out[J * SQ + t0_ * 128 : J * SQ + t1_ * 128, :]
                            .rearrange("(t p) h -> p t h", p=128),
                            ob[:, t0_ * H : t1_ * H]
                            .rearrange("p (t h) -> p t h", t=t1_ - t0_),
                        )

            # schedule: proj0/proj1 are DMA-paced (emit directly); proj2/3
            # interleave into att1/att2; out_{J-1} lands after pair 1
            qk0, vv0, p0 = proj_start(0)
            for em in p0:
                em()
            proj_finish(0, vv0)
            vtrans(0)
            att(0)
            qk1, vv1, p1 = proj_start(1)
            for em in p1:
                em()
            proj_finish(1, vv1)
            vtrans(1)
            qk2, vv2, p2 = proj_start(2)
            att(1, inter=p2, after_pair1=lambda: outp(0))
            proj_finish(2, vv2)
            vtrans(2)
            qk3, vv3, p3 = proj_start(3)
            att(2, inter=p3, after_pair1=lambda: outp(1))
            proj_finish(3, vv3)
            vtrans(3)
            att(3, after_pair1=lambda: outp(2))
            outp(NQ - 1)

    nc.finalize()
    return nc


def _host_prep(x, Wq, bq, Wk, bk, Wv, bv):
    """Layout-only host prep: shard x by batch + pack weight operands."""
    import ml_dtypes

    f32 = np.float32
    bf16 = ml_dtypes.bfloat16
    wqk = np.concatenate([Wq, Wk], axis=1).reshape(ND, 128, 2 * H)
    wv1 = np.concatenate([Wv, np.zeros((D, 1), f32)], axis=1).reshape(
        ND, 128, H1
    )
    wchunks = np.concatenate([wqk, wv1], axis=2)        # [ND, 128, CW]
    wqkv_w = wchunks.transpose(1, 0, 2).reshape(128, ND * CW)
    bqk = np.concatenate([bq, bk])[:, None]             # [128, 1]
    bv1 = np.zeros((128, 1), f32)
    bv1[:H1, 0] = np.concatenate([bv, np.ones((1,), f32)])
    wqkv = np.ascontiguousarray(
        np.concatenate([bqk, bv1, wqkv_w], axis=1), dtype=bf16
    )
    kk = np.arange(128)[:, None]
    identb = np.eye(128, dtype=bf16)
    # tneg in S^T [k-partition, q-col] orientation: -1e30 where q < k
    tneg = np.where(kk > np.arange(128)[None, :], -1e30, 0.0).astype(bf16)
    idt = np.ascontiguousarray(np.concatenate([identb, tneg], axis=1), dtype=bf16)
    common = {"wqkv": wqkv, "idt": idt}
    in_maps = []
    for b in range(B):
        m = dict(common)
        # xTp[J, p, c*SQ+s] = x[b][SQ*J+s, 128*c+p]
        m["xTp"] = np.ascontiguousarray(
            x[b].reshape(NQ, SQ, ND, 128).transpose(0, 3, 2, 1), dtype=bf16
        ).reshape(NQ, 128, ND * SQ)
        in_maps.append(m)
    return in_maps


def run(x, Wq, bq, Wk, bk, Wv, bv, trace=False):
    from concourse.bass_utils import run_bass_kernel_spmd

    if "nc" not in _CACHE:
        _CACHE["nc"] = _build_nc()
    nc = _CACHE["nc"]
    in_maps = _host_prep(
        np.asarray(x), np.asarray(Wq), np.asarray(bq), np.asarray(Wk),
        np.asarray(bk), np.asarray(Wv), np.asarray(bv),
    )
    res = run_bass_kernel_spmd(
        nc, in_maps, core_ids=list(range(N_CORES)), trace=trace
    )
    outs = np.stack([res.results[c]["out"] for c in range(N_CORES)], axis=0)
    return outs.astype(np.float32), res


def kernel(x, Wq, bq, Wk, bk, Wv, bv):
    outs, _ = run(x, Wq, bq, Wk, bk, Wv, bv, trace=False)
    return outs


# revision 20
# speedup vs baseline: 1.0404x; 1.0404x over previous
"""Trainium2 Bass kernel: single-head causal self-attention.

Problem: x[B=8, S=2048, D=1024], Wq/Wk/Wv[D, H=64], bq/bk/bv[H].
    q = x@Wq+bq; k = x@Wk+bk; v = x@Wv+bv
    out = softmax(causal(q k^T) / sqrt(H)) @ v

Sharding: batch -> 8 NeuronCores (data parallel, no collectives).

Per-core strategy (~62-64us vs ~80us baseline, warm):
  - all matmuls bf16; scores built transposed: S^T [128 k, 512 q] =
    K_i Q^T in PSUM; exp (scale=1/8, bf16 out) gives P^T tiles = the
    moving operand of O^T = (V|1)^T P^T; the (V|1) ones-column yields
    the softmax denominator as an extra output row
  - head: first matmul gated on ~113 KiB across 3 HW queues; pieces
    issued in need-order alternating sync/scalar (the only HWDGE
    engines; all in-flight transfers share HBM, so piece count is
    kept low to get the J1 triggers out early); 1 MiB J-chunks split
    in half across the two engines
  - att emits batches of two pairs: 4 score matmuls, then the 4 PV
    matmuls of the PREVIOUS batch. The dense PE stream sustains the
    DVFS boost clock (~2x) through the whole kernel and halves the
    group-boundary LDWEIGHTS exposure; PVs lagging a batch means
    they never wait on their exp
  - proj(J+1) piece closures (qk c0-7, qk-finish, vv c0-7) interleave
    4-per-batch into att(J)'s exp gaps; qk-finish (bias + K^T move)
    sits between qk and vv so KT0 is ready early
  - causal mask: fully-masked q-ranges trimmed from matmuls AND exp;
    the 128x128 triangle gets -1e30 DVE-added on PSUM (off the PE)
  - K^T to partitions 0:64: PE selector (shared vtr/ksel bank) +
    scalar copy for J=0,1; SBUF->SBUF sync DMA for J=2,3
  - out path is all-vector (scalar stays exp-only); the last chunk
    stores in two halves on sync and scalar
"""

import sys

sys.path.insert(0, "/opt/trn_rl_repo")

import numpy as np

B, S, D, H = 8, 2048, 1024, 64
N_CORES = 8
SQ = 512            # q chunk (PSUM bank / fp32 moving max)
NQ = S // SQ        # 4
ND = D // 128       # 8 contraction chunks for projections
H1 = H + 1          # V plus ones column
CW = 2 * H + H1     # packed weight cols per chunk: wq|wk|wv1 = 193
TRIM = True         # skip fully-masked q-ranges in diagonal blocks

_CACHE = {}


def _build_nc():
    import concourse.tile as tile
    from concourse import bacc, mybir

    f32 = mybir.dt.float32
    bf16 = mybir.dt.bfloat16
    AF = mybir.ActivationFunctionType
    ALU = mybir.AluOpType

    nc = bacc.Bacc(None, target_bir_lowering=False)
    xTp = nc.dram_tensor("xTp", [NQ, 128, ND * SQ], bf16, kind="ExternalInput")
    # wqkv: [bqk | bv1 | chunk c: (wq|wk 128 cols, wv1 65 cols)]
    wqkv = nc.dram_tensor("wqkv", [128, 2 + ND * CW], bf16, kind="ExternalInput")
    # idt: [identb(128) | tneg(128)]
    idt = nc.dram_tensor("idt", [128, 256], bf16, kind="ExternalInput")
    out = nc.dram_tensor("out", [S, H], f32, kind="ExternalOutput")

    with tile.TileContext(nc) as tc:
        from contextlib import ExitStack

        with ExitStack() as ctx:
            const = ctx.enter_context(tc.tile_pool(name="const", bufs=1))
            sb = ctx.enter_context(tc.tile_pool(name="sb", bufs=1))
            pt_pool = ctx.enter_context(tc.tile_pool(name="pt", bufs=6))
            o_pool = ctx.enter_context(tc.tile_pool(name="o", bufs=2))
            ps = ctx.enter_context(tc.tile_pool(name="ps", bufs=1, space="PSUM"))

            wq_sb = const.tile([128, 2 + ND * CW], bf16)
            idt_sb = const.tile([128, 256], bf16)
            identb_sb = idt_sb[:, 0:128]
            tneg_sb = idt_sb[:, 128:256]

            def wchunk(c):       # wq|wk stationary for chunk c
                return wq_sb[:, 2 + c * CW : 2 + c * CW + 2 * H]

            def vchunk(c):       # wv|ones-bias stationary for chunk c
                return wq_sb[:, 2 + c * CW + 2 * H : 2 + (c + 1) * CW]

            bf = const.tile([128, 2], f32)
            bqk_sb = bf[:, 0:1]
            bv1_sb = bf[:H1, 1:2]

            xt = {}          # J -> [128, ND*SQ] bf16 (c-chunk at cols c*SQ)
            QKT = {}         # J -> [128, SQ] bf16 (Q^T rows 0:64, K^T 64:128)
            KT0 = {}         # J -> [64, SQ] bf16 at base partition 0
            Vones = {}       # J -> [128, 4*66] bf16 ((V|1) rows, stride 66)
            VT1 = {}         # J -> [H1, SQ] bf16 (V^T plus ones row)
            OT = {}

            # ---- DMA issue order: need-ordered, low piece count ----
            with nc.named_scope("load"):
                for J in range(NQ):
                    t_x = sb.tile([128, ND * SQ], bf16, tag=f"x{J}")
                    xt[J] = t_x
                x0 = xt[0]
                HP = ND * SQ // 2   # half of a J chunk
                nc.sync.dma_start(wq_sb[:, : 2 + CW], wqkv[:, : 2 + CW])
                nc.scalar.dma_start(x0[:, 256:512], xTp[0, :, 256:512])
                nc.sync.dma_start(x0[:, 0:256], xTp[0, :, 0:256])
                nc.scalar.dma_start(x0[:, 512:1024], xTp[0, :, 512:1024])
                nc.sync.dma_start(
                    wq_sb[:, 2 + CW : 2 + 4 * CW], wqkv[:, 2 + CW : 2 + 4 * CW]
                )
                nc.sync.dma_start(x0[:, 1024:2048], xTp[0, :, 1024:2048])
                nc.sync.dma_start(idt_sb[:], idt[:, :])
                nc.scalar.dma_start(x0[:, 2048:3072], xTp[0, :, 2048:3072])
                nc.sync.dma_start(x0[:, 3072:4096], xTp[0, :, 3072:4096])
                nc.scalar.dma_start(wq_sb[:, 2 + 4 * CW :], wqkv[:, 2 + 4 * CW :])
                nc.scalar.dma_start(xt[1][:, :HP], xTp[1, :, :HP])
                nc.sync.dma_start(xt[1][:, HP:], xTp[1, :, HP:])
                nc.scalar.dma_start(xt[2][:, :HP], xTp[2, :, :HP])
                nc.sync.dma_start(xt[2][:, HP:], xTp[2, :, HP:])
                nc.scalar.dma_start(xt[3][:, :HP], xTp[3, :, :HP])
                nc.sync.dma_start(xt[3][:, HP:], xTp[3, :, HP:])
                nc.vector.tensor_copy(bf[:], wq_sb[:, 0:2])

            def proj_start(J):
                """Alloc proj PSUM tiles; return (qk, vv, piece closures).
                Pieces: qk c0..c7, qk-finish (bias + K^T move), vv c0..c7.
                Emitting them in order (possibly interleaved into att)
                reproduces the projection."""
                qk = ps.tile([128, SQ], f32, tag="proj", bufs=2)
                vv = ps.tile([H1, SQ], f32, tag="proj", bufs=2)

                def mk_qk(c):
                    def em():
                        if J == 0 and c == 0:
                            nc.tensor.matmul(
                                qk[:, 0:256], wchunk(0), xt[0][:, 0:256],
                                start=True, stop=False,
                            )
                            # bytes still pending-zero from the first piece
                            nc.tensor.matmul(
                                qk[:, 256:512], wchunk(0), xt[0][:, 256:512],
                                start=False, stop=False,
                            )
                        else:
                            nc.tensor.matmul(
                                qk[:], wchunk(c),
                                xt[J][:, c * SQ : (c + 1) * SQ],
                                start=(c == 0), stop=(c == ND - 1),
                            )
                    return em

                def fin_qk():
                    with nc.named_scope(f"projf{J}"):
                        qkt = sb.tile([128, SQ], bf16, tag=f"qkt{J}")
                        nc.vector.tensor_scalar_add(
                            qkt[H:, :], qk[H:, :], bqk_sb[H:, :]
                        )
                        nc.vector.tensor_scalar_add(
                            qkt[:H, :], qk[:H, :], bqk_sb[:H, :]
                        )
                        QKT[J] = qkt
                        kt0 = sb.tile([H, SQ], bf16, tag=f"kt0{J}")
                        if J < 2:
                            ksel = ps.tile([H, SQ], f32, tag="vtr", bufs=1)
                            nc.tensor.matmul(
                                ksel[:], identb_sb[H:, H:], qkt[H:, :],
                                start=True, stop=True,
                            )
                            nc.scalar.activation(kt0[:], ksel[:], AF.Copy)
                        else:
                            nc.sync.dma_start(kt0[:], qkt[H:, :])
                        KT0[J] = kt0

                def mk_vv(c):
                    def em():
                        nc.tensor.matmul(
                            vv[:], vchunk(c), xt[J][:, c * SQ : (c + 1) * SQ],
                            start=(c == 0), stop=(c == ND - 1),
                        )
                    return em

                pieces = [mk_qk(c) for c in range(ND)]
                pieces.append(fin_qk)
                pieces += [mk_vv(c) for c in range(ND)]
                return qk, vv, pieces

            def proj_finish(J, vv):
                with nc.named_scope(f"projv{J}"):
                    vt1 = sb.tile([H1, SQ], bf16, tag=f"vt1{J}")
                    nc.vector.tensor_scalar_add(vt1[:], vv[:], bv1_sb[:])
                    VT1[J] = vt1

            def vtrans(J):
                with nc.named_scope(f"vtr{J}"):
                    vt1 = VT1[J]
                    pst = ps.tile([128, 4 * 66], bf16, tag="vtr", bufs=1)
                    for tt in range(4):
                        nc.tensor.transpose(
                            pst[:, tt * 66 : tt * 66 + H1],
                            vt1[:, tt * 128 : (tt + 1) * 128],
                            identb_sb[:H1, :H1],
                        )
                    vo = sb.tile([128, 4 * 66], bf16, tag=f"vo{J}")
                    nc.vector.tensor_copy(
                        vo[:].rearrange("p (t u) -> p t u", t=4)[:, :, 0:H1],
                        pst[:].rearrange("p (t u) -> p t u", t=4)[:, :, 0:H1],
                    )
                    Vones[J] = vo

            def att(J, inter=(), after_pair1=None):
                """PV pairs lag one pair behind scores; `inter` closures
                (next proj's pieces) are emitted 2 per pair from pair 1
                on, remainder after the loop."""
                last = J == NQ - 1
                inter = list(inter)
                ip = 0
                with nc.named_scope(f"att{J}"):
                    ot = ps.tile([H1, SQ], f32, tag="ot", bufs=1)
                    OT[J] = ot
                    nhalf = 2 * (J + 1)

                    def pv_pair(ii, pt):
                        diag = ii >= 2 * J
                        for h2 in range(2):
                            i = 2 * ii + h2
                            r = i - 4 * J
                            q0 = 128 * r if (TRIM and diag and r > 0) else 0
                            vo = Vones[i // 4][:, (i % 4) * 66 : (i % 4) * 66 + H1]
                            nc.tensor.matmul(
                                ot[:, q0:], vo,
                                pt[:, h2 * SQ + q0 : (h2 + 1) * SQ],
                                start=(i == 0),
                                stop=(i == 4 * (J + 1) - 1),
                            )

                    def score_pair(ii):
                        diag = ii >= 2 * J
                        st = ps.tile([128, 2 * SQ], f32, tag="st", bufs=2)
                        for h2 in range(2):
                            i = 2 * ii + h2
                            r = i - 4 * J
                            q0 = 128 * r if (TRIM and diag and r > 0) else 0
                            nc.tensor.matmul(
                                st[:, h2 * SQ + q0 : (h2 + 1) * SQ],
                                KT0[i // 4][:, (i % 4) * 128 : (i % 4 + 1) * 128],
                                QKT[J][:H, q0:],
                                start=True,
                                stop=True,
                            )
                            if diag:
                                nc.vector.tensor_tensor(
                                    st[:, h2 * SQ + 128 * r :
                                       h2 * SQ + 128 * (r + 1)],
                                    st[:, h2 * SQ + 128 * r :
                                       h2 * SQ + 128 * (r + 1)],
                                    tneg_sb[:],
                                    ALU.add,
                                )
                        pt = pt_pool.tile([128, 2 * SQ], bf16, tag="pt")
                        if TRIM and diag and ii == 2 * J:
                            nc.scalar.activation(
                                pt[:, 0:512], st[:, 0:512], AF.Exp, scale=0.125
                            )
                            nc.scalar.activation(
                                pt[:, 640:1024], st[:, 640:1024],
                                AF.Exp, scale=0.125,
                            )
                        elif TRIM and diag and ii == 2 * J + 1:
                            nc.scalar.activation(
                                pt[:, 256:512], st[:, 256:512],
                                AF.Exp, scale=0.125,
                            )
                            nc.scalar.activation(
                                pt[:, 896:1024], st[:, 896:1024],
                                AF.Exp, scale=0.125,
                            )
                        else:
                            nc.scalar.activation(
                                pt[:], st[:], AF.Exp, scale=0.125
                            )
                        return pt

                    # batch two pairs per PE group (4 score matmuls, then
                    # 4 PV matmuls of the previous batch): halves the
                    # group-boundary LDWEIGHTS exposure; PVs lag a batch
                    pend = []
                    for b in range(nhalf // 2):
                        cur = []
                        for ii in (2 * b, 2 * b + 1):
                            cur.append((ii, score_pair(ii)))
                        for p in pend:
                            pv_pair(*p)
                        pend = cur
                        if b == 0 and after_pair1 is not None:
                            after_pair1()
                        if b >= 1:
                            for _ in range(4):
                                if ip < len(inter):
                                    inter[ip]()
                                    ip += 1
                    for p in pend:
                        pv_pair(*p)
                    while ip < len(inter):
                        inter[ip]()
                        ip += 1

            def outp(J):
                # normalize + store rows 512J..512J+511
                last = J == NQ - 1
                halves = [(0, 2), (2, 4)] if last else [(0, 4)]
                # all-vector: scalar stays exp-only in the att windows
                use_vec = True
                with nc.named_scope(f"out{J}"):
                    ots = sb.tile([H1, SQ], bf16, tag=f"ots{J}")
                    ob = o_pool.tile([128, 4 * H], f32, tag="ob")
                    po = ps.tile([128, 4 * 66], bf16, tag="vtr", bufs=1)
                    rc = o_pool.tile([128, 4], f32, tag="rc")
                    for bi, (t0_, t1_) in enumerate(halves):
                        src = OT[J][:, t0_ * 128 : t1_ * 128]
                        if use_vec:
                            nc.vector.tensor_copy(
                                ots[:, t0_ * 128 : t1_ * 128], src
                            )
                        else:
                            nc.scalar.activation(
                                ots[:, t0_ * 128 : t1_ * 128], src, AF.Copy
                            )
                        for tt in range(t0_, t1_):
                            nc.tensor.transpose(
                                po[:, tt * 66 : tt * 66 + H1],
                                ots[:, tt * 128 : (tt + 1) * 128],
                                identb_sb[:H1, :H1],
                            )
                        nc.vector.reciprocal(
                            rc[:, t0_:t1_],
                            po[:, t0_ * 66 + H : t1_ * 66 : 66],
                        )
                        for tt in range(t0_, t1_):
                            if use_vec:
                                nc.vector.tensor_scalar_mul(
                                    ob[:, tt * H : (tt + 1) * H],
                                    po[:, tt * 66 : tt * 66 + H],
                                    rc[:, tt : tt + 1],
                                )
                            else:
                                nc.scalar.activation(
                                    ob[:, tt * H : (tt + 1) * H],
                                    po[:, tt * 66 : tt * 66 + H],
                                    AF.Copy,
                                    scale=rc[:, tt : tt + 1],
                                )
                        eng = nc.scalar if (last and bi == 1) else nc.sync
                        eng.dma_start(
                            out[J * SQ + t0_ * 128 : J * SQ + t1_ * 128, :]
                            .rearrange("(t p) h -> p t h", p=128),
                            ob[:, t0_ * H : t1_ * H]
                            .rearrange("p (t h) -> p t h", t=t1_ - t0_),
                        )

            # schedule: proj0/proj1 are DMA-paced (emit directly); proj2/3
            # interleave into att1/att2; out_{J-1} lands after pair 1
            qk0, vv0, p0 = proj_start(0)
            for em in p0:
                em()
            proj_finish(0, vv0)
            vtrans(0)
            att(0)
            qk1, vv1, p1 = proj_start(1)
            for em in p1:
                em()
            proj_finish(1, vv1)
            vtrans(1)
            qk2, vv2, p2 = proj_start(2)
            att(1, inter=p2, after_pair1=lambda: outp(0))
            proj_finish(2, vv2)
            vtrans(2)
            qk3, vv3, p3 = proj_start(3)
            att(2, inter=p3, after_pair1=lambda: outp(1))
            proj_finish(3, vv3)
            vtrans(3)
            att(3, after_pair1=lambda: outp(2))
            outp(NQ - 1)

    nc.finalize()
    return nc


def _host_prep(x, Wq, bq, Wk, bk, Wv, bv):
    """Layout-only host prep: shard x by batch + pack weight operands."""
    import ml_dtypes

    f32 = np.float32
    bf16 = ml_dtypes.bfloat16
    wqk = np.concatenate([Wq, Wk], axis=1).reshape(ND, 128, 2 * H)
    wv1 = np.concatenate([Wv, np.zeros((D, 1), f32)], axis=1).reshape(
        ND, 128, H1
    )
    wchunks = np.concatenate([wqk, wv1], axis=2)        # [ND, 128, CW]
    wqkv_w = wchunks.transpose(1, 0, 2).reshape(128, ND * CW)
    bqk = np.concatenate([bq, bk])[:, None]             # [128, 1]
    bv1 = np.zeros((128, 1), f32)
    bv1[:H1, 0] = np.concatenate([bv, np.ones((1,), f32)])
    wqkv = np.ascontiguousarray(
        np.concatenate([bqk, bv1, wqkv_w], axis=1), dtype=bf16
    )
    kk = np.arange(128)[:, None]
    identb = np.eye(128, dtype=bf16)
    # tneg in S^T [k-partition, q-col] orientation: -1e30 where q < k
    tneg = np.where(kk > np.arange(128)[None, :], -1e30, 0.0).astype(bf16)
    idt = np.ascontiguousarray(np.concatenate([identb, tneg], axis=1), dtype=bf16)
    common = {"wqkv": wqkv, "idt": idt}
    in_maps = []
    for b in range(B):
        m = dict(common)
        # xTp[J, p, c*SQ+s] = x[b][SQ*J+s, 128*c+p]
        m["xTp"] = np.ascontiguousarray(
            x[b].reshape(NQ, SQ, ND, 128).transpose(0, 3, 2, 1), dtype=bf16
        ).reshape(NQ, 128, ND * SQ)
        in_maps.append(m)
    return in_maps


def run(x, Wq, bq, Wk, bk, Wv, bv, trace=False):
    from concourse.bass_utils import run_bass_kernel_spmd

    if "nc" not in _CACHE:
        _CACHE["nc"] = _build_nc()
    nc = _CACHE["nc"]
    in_maps = _host_prep(
        np.asarray(x), np.asarray(Wq), np.asarray(bq), np.asarray(Wk),
        np.asarray(bk), np.asarray(Wv), np.asarray(bv),
    )
    res = run_bass_kernel_spmd(
        nc, in_maps, core_ids=list(range(N_CORES)), trace=trace
    )
    outs = np.stack([res.results[c]["out"] for c in range(N_CORES)], axis=0)
    return outs.astype(np.float32), res


def kernel(x, Wq, bq, Wk, bk, Wv, bv):
    outs, _ = run(x, Wq, bq, Wk, bk, Wv, bv, trace=False)
    return outs
